# revision 23
# baseline (speedup 1.0000x reference)
"""Trainium2 Bass kernel for MinibatchDiscrimination.

Math (reference):
    M = (x @ T.reshape(512, 320)).reshape(1024, 64, 5)
    dist[i, j, f] = sum_k |M[i, f, k] - M[j, f, k]|
    out[i, f] = sum_j exp(-dist[i, j, f])            # (1024, 64)

Device strategy (8 cores, SPMD, rows of the output sharded):
  Every core holds MT = (T2r)^T @ x^T laid out as (320 rows = k*64+f,
  1024 cols = j) in SBUF (bf16), where its own 128 output rows are MT
  columns 0..127 (host rotates x's rows per core).  Per output row i:

    relu trick:  |d| = 2*relu(d) - d   with  d_k = MT_k[f, j] - MT_k[f, i]
    sum_k d_k   = SM[f, j] - SM[f, i]  with  SM = sum_k MT_k
    => dist = 2*( sum_k relu(d_k) - SM_j/2 ) - (-SM_i)

    - DVE:  3x tensor_scalar (subtract, max 0) over the three 128-row
      chunks of MT -> relu tiles (bf16, 4x perf mode).
    - PE:   selection-matmul k-sum of the relu tiles into one PSUM bank
      (128 = (f, j-half) partitions x 512) + one identity matmul
      streaming the static -SM/2 plane.
    - ACT:  exp(-2*PSUM + bias=-SM_i) with accum_out = j-sum -> one
      column of the (128, 128) output accumulator.

  Prologue engineering (the loop above is engine-floor bound; the
  measured waste was a ~30us prologue):
    - inputs DMA'd in 128-row contraction chunks across 3 queues so the
      first MT matmul starts ~1.5us in instead of waiting for the full
      transfer;
    - SM is produced by the same matmuls as the k4 MT chunk via an
      augmented stationary [T2sum | T2k4] (host ships T2sum = sum_k
      T2_k), instead of 6 extra selection matmuls;
    - per-i scalar tables (mts*, negsm) are built only for the 128 own
      columns.

  Host: out[i, f] = acc[f, i] + acc[f + 64, i], gather 8 cores.
"""

import numpy as np
import ml_dtypes

import concourse.bass as bass
import concourse.bacc as bacc
import concourse.mybir as mybir
import concourse.tile as tile
from concourse import bass_utils

BF16 = ml_dtypes.bfloat16

N, IN_F, OUT_F, KD = 1024, 512, 64, 5
NCORES = 8
ROWS = N // NCORES          # 128 output rows per core
R2 = 384                    # t2all cols: 256 (k0..k3) + 64 (T2sum) + 64 (k4)
FCH = IN_F // 128           # 4 contraction chunks for the MT matmul
JH = N // 2                 # 512, j-half width (PSUM bank)

_COMPILED = None            # compile once per process


def _build_program():
    nc = bacc.Bacc("TRN2", target_bir_lowering=False, debug=False,
                   num_devices=NCORES)
    dt = mybir.dt

    xt_d = nc.dram_tensor("xt", [IN_F, N], dt.bfloat16, kind="ExternalInput").ap()
    t2_d = nc.dram_tensor("t2a", [IN_F, R2], dt.bfloat16, kind="ExternalInput").ap()
    sel_d = nc.dram_tensor("sel", [128, 64], dt.bfloat16, kind="ExternalInput").ap()
    dup_d = nc.dram_tensor("dup", [64, 128], dt.bfloat16, kind="ExternalInput").ap()
    idn_d = nc.dram_tensor("idn", [128, 128], dt.bfloat16, kind="ExternalInput").ap()
    acc_d = nc.dram_tensor("acc", [128, 128], dt.float32, kind="ExternalOutput").ap()

    with tile.TileContext(nc) as tc:
        with (
            tc.tile_pool(name="persist", bufs=1) as pp,
            tc.tile_pool(name="relu", bufs=10) as rp,
            tc.tile_pool(name="psA", bufs=2, space="PSUM") as psA,
            tc.tile_pool(name="psB", bufs=6, space="PSUM") as psB,
        ):
            # ---- chunked input DMAs (3 queues) so PE starts early ----------
            xt_big = pp.tile([128, FCH * N], dt.bfloat16, tag="xtb", name="xt_big")
            t2_big = pp.tile([128, FCH * R2], dt.bfloat16, tag="t2b", name="t2_big")
            xt_sb = [xt_big[:, fc * N:(fc + 1) * N] for fc in range(FCH)]
            t2_sb = [t2_big[:, fc * R2:(fc + 1) * R2] for fc in range(FCH)]
            for fc in range(FCH):
                psl = slice(fc * 128, (fc + 1) * 128)
                q = nc.sync if fc % 2 == 0 else nc.gpsimd
                q.dma_start(xt_sb[fc], xt_d[psl, :])
                nc.scalar.dma_start(t2_sb[fc], t2_d[psl, :])
            sel_sb = pp.tile([128, 64], dt.bfloat16, tag="sel", name="sel_sb")
            nc.gpsimd.dma_start(sel_sb[:], sel_d[:])
            dup_sb = pp.tile([64, 128], dt.bfloat16, tag="dup", name="dup_sb")
            nc.gpsimd.dma_start(dup_sb[:], dup_d[:])
            idn_sb = pp.tile([128, 128], dt.bfloat16, tag="idn", name="idn_sb")
            nc.gpsimd.dma_start(idn_sb[:], idn_d[:])

            # ---- MT = t2^T @ xt  (320, 1024) in 3 chunk tiles, bf16 ---------
            # chunk 0: rows 0..127 (k0,k1), chunk 1: 128..255 (k2,k3),
            # k4 packed as (f, j-half) x 512 (mtb2p).  SM rides along in the
            # k4 matmuls via the [T2sum | T2k4] stationary columns 256..384.
            mtb = [
                pp.tile([128, N], dt.bfloat16, tag="mtb0", name="mtb0"),
                pp.tile([128, N], dt.bfloat16, tag="mtb1", name="mtb1"),
            ]
            mtb2p = pp.tile([128, JH], dt.bfloat16, tag="mtb2p", name="mtb2p")
            smb = pp.tile([64, N], dt.bfloat16, tag="smb", name="smb")
            stage = pp.tile([128, JH], dt.bfloat16, tag="stage", name="stage")
            # fp32 per-i scalar tables; only the core's own 128 columns.
            mts = [
                pp.tile([128, ROWS], dt.float32, tag="mts0", name="mts0"),
                pp.tile([128, ROWS], dt.float32, tag="mts1", name="mts1"),
            ]
            mts2p = pp.tile([128, ROWS], dt.float32, tag="mts2p", name="mts2p")

            # rc-waves, fc-inner so each wave streams chunks as they land.
            for rc in (0, 1):
                rsl = slice(rc * 128, rc * 128 + 128)
                pss = [psA.tile([128, JH], dt.float32, tag="psA", name="psA")
                       for _ in range(2)]
                for fc in range(FCH):
                    for h in range(2):
                        nc.tensor.matmul(
                            pss[h][:], lhsT=t2_sb[fc][:, rsl],
                            rhs=xt_sb[fc][:, h * JH:(h + 1) * JH],
                            start=(fc == 0), stop=(fc == FCH - 1),
                        )
                for h in range(2):
                    nc.scalar.copy(mtb[rc][:, h * JH:(h + 1) * JH], pss[h][:])
                nc.vector.tensor_copy(mts[rc][:], mtb[rc][:, 0:ROWS])

            # k4 + SM waves: stationary cols 256:384 = [T2sum | T2k4]:
            # out partitions 0..63 = SM, 64..127 = M_k4.
            for h in range(2):
                jsl = slice(h * JH, (h + 1) * JH)
                ps = psA.tile([128, JH], dt.float32, tag="psA", name="psA")
                for fc in range(FCH):
                    nc.tensor.matmul(ps[:], lhsT=t2_sb[fc][:, 256:384],
                                     rhs=xt_sb[fc][:, jsl],
                                     start=(fc == 0), stop=(fc == FCH - 1))
                nc.scalar.copy(smb[:, jsl], ps[0:64, :])
                if h == 0:
                    nc.scalar.copy(stage[64:128, :], ps[64:128, :])
                else:
                    nc.scalar.copy(mtb2p[64:128, :], ps[64:128, :])
            # partition shift 64..127 -> 0..63 for the h0 k4 plane (DMA can
            # cross partitions; engines cannot).
            nc.sync.dma_start(mtb2p[0:64, :], stage[64:128, :])

            # mts2p[f + 64h, i] = mtb2p[f, i] for both h (exact upcast via
            # the 64->128 duplicator matmul; bf16 -> fp32 is exact).
            psd = psA.tile([128, JH], dt.float32, tag="psA", name="psA")
            nc.tensor.matmul(psd[:, 0:ROWS], lhsT=dup_sb[:],
                             rhs=mtb2p[0:64, 0:ROWS], start=True, stop=True)
            nc.scalar.copy(mts2p[:], psd[:, 0:ROWS])

            # ---- -SM/2 packed (f, h) + per-i bias, both smb-derived so the
            # self-term cancels exactly (bf16(-x/2) and fp32(-x) are exact).
            smp = pp.tile([128, JH], dt.bfloat16, tag="smp", name="smp")
            negsm = pp.tile([128, ROWS], dt.float32, tag="negsm", name="negsm")
            for h in range(2):
                jsl = slice(h * JH, (h + 1) * JH)
                ps = psA.tile([128, JH], dt.float32, tag="psA", name="psA")
                nc.tensor.matmul(ps[:], lhsT=dup_sb[:], rhs=smb[:, jsl],
                                 start=True, stop=True)
                nc.scalar.mul(smp[h * 64:h * 64 + 64, :],
                              ps[h * 64:h * 64 + 64, :], -0.5)
                if h == 0:
                    nc.scalar.activation(negsm[:], ps[:, 0:ROWS],
                                         mybir.ActivationFunctionType.Copy,
                                         bias=0.0, scale=-1.0)

            # ---- output accumulator + ACT scratch ---------------------------
            outacc = pp.tile([128, ROWS], dt.float32, tag="outacc", name="outacc")
            esc = psA.tile([128, JH], dt.float32, tag="psA", name="psA")



            # ---- main loop over the core's 128 output rows ------------------
            for i in range(ROWS):
                r0 = rp.tile([128, N], dt.bfloat16, tag="r0", name="r0")
                r1 = rp.tile([128, N], dt.bfloat16, tag="r1", name="r1")
                c2tt = rp.tile([128, JH], dt.bfloat16, tag="c2t", name="c2t")
                c2t = c2tt[:]
                nc.vector.tensor_scalar(
                    out=r0[:], in0=mtb[0][:], scalar1=mts[0][:, i:i + 1],
                    scalar2=0.0, op0=mybir.AluOpType.subtract,
                    op1=mybir.AluOpType.max)
                nc.vector.tensor_scalar(
                    out=r1[:], in0=mtb[1][:], scalar1=mts[1][:, i:i + 1],
                    scalar2=0.0, op0=mybir.AluOpType.subtract,
                    op1=mybir.AluOpType.max)
                nc.vector.tensor_scalar(
                    out=c2t, in0=mtb2p[:], scalar1=mts2p[:, i:i + 1],
                    scalar2=0.0, op0=mybir.AluOpType.subtract,
                    op1=mybir.AluOpType.max)

                # one PSUM bank, partitions (f, h): per-h selection-matmul
                # groups on disjoint partition ranges, then two full-height
                # identity matmuls add the packed k4 relu and the static
                # -SM/2.  The sim's flat group-check conflates the per-h
                # groups; pending-zero semantics stay per-partition exact.
                ps = psB.tile([128, JH], dt.float32, tag="psB", name="psB")
                for h in range(2):
                    jsl = slice(h * JH, (h + 1) * JH)
                    osl = ps[h * 64:h * 64 + 64, :]
                    nc.tensor.matmul(osl, lhsT=sel_sb[:], rhs=r0[:, jsl],
                                     start=True, stop=False,
                                     skip_group_check=True)
                    nc.tensor.matmul(osl, lhsT=sel_sb[:], rhs=r1[:, jsl],
                                     start=False, stop=False,
                                     skip_group_check=True)
                nc.tensor.matmul(ps[:], lhsT=idn_sb[:], rhs=c2t,
                                 start=False, stop=False,
                                 skip_group_check=True)
                nc.tensor.matmul(ps[:], lhsT=idn_sb[:], rhs=smp[:],
                                 start=False, stop=True,
                                 skip_group_check=True)

                nc.scalar.activation(
                    esc[:], ps[:], mybir.ActivationFunctionType.Exp,
                    bias=negsm[:, i:i + 1], scale=-2.0,
                    accum_out=outacc[:, i:i + 1])

            nc.sync.dma_start(acc_d[:], outacc[:])

    nc.compile()
    return nc


def _host_inputs(x, T):
    """Full-input host prep -> per-core input maps."""
    xt = np.ascontiguousarray(x.T).astype(BF16)                  # (512, 1024)
    t2r = np.ascontiguousarray(
        T.transpose(0, 2, 1).reshape(IN_F, OUT_F * KD)).astype(BF16)
    t2sum = (t2r[:, 0:64].astype(np.float32) + t2r[:, 64:128]
             + t2r[:, 128:192] + t2r[:, 192:256]
             + t2r[:, 256:320]).astype(BF16)
    t2a = np.concatenate([t2r[:, 0:256], t2sum, t2r[:, 256:320]], axis=1)

    f_idx = np.arange(64)
    sel = (np.arange(128)[:, None] % 64 == f_idx[None, :]).astype(BF16)
    dup = (np.arange(128)[None, :] % 64 == np.arange(64)[:, None]).astype(BF16)
    idn = np.eye(128, dtype=np.float32).astype(BF16)

    in_maps = []
    for c in range(NCORES):
        xt_c = np.roll(xt, -ROWS * c, axis=1)
        in_maps.append({"xt": np.ascontiguousarray(xt_c),
                        "t2a": np.ascontiguousarray(t2a),
                        "sel": sel, "dup": dup, "idn": idn})
    return in_maps


def _assemble(results):
    outs = []
    for c in range(NCORES):
        acc = results[c]["acc"]                      # (128, 128) f32
        outs.append((acc[:64, :] + acc[64:, :]).T)   # (128 rows, 64 f)
    return np.ascontiguousarray(np.concatenate(outs, axis=0), dtype=np.float32)


def _ensure_ntff_hook():
    """The agent image's antenv lacks axon_hooks; shim it so trace=True
    works (bass_utils imports antenv.axon_hooks unconditionally)."""
    import sys
    import types
    try:
        from antenv import axon_hooks  # noqa: F401
        return
    except ImportError:
        pass
    mod = types.ModuleType("antenv.axon_hooks")
    holder = [None]
    mod.set_axon_ntff_profile_hook = lambda h: holder.__setitem__(0, h)
    mod.get_axon_ntff_profile_hook = lambda: holder[0]
    import antenv
    antenv.axon_hooks = mod
    sys.modules["antenv.axon_hooks"] = mod
    try:
        from trn_agent_boot.trn_boot import _ntff_profile_via_ctypes
        h = _ntff_profile_via_ctypes("/opt/axon/libaxon_pjrt.so")
        if h is not None:
            mod.set_axon_ntff_profile_hook(h)
    except Exception:
        pass


def _get_compiled():
    global _COMPILED
    if _COMPILED is None:
        _COMPILED = _build_program()
    return _COMPILED


def kernel(x, T, _trace=False):
    if _trace:
        _ensure_ntff_hook()
    nc = _get_compiled()
    in_maps = _host_inputs(np.asarray(x, dtype=np.float32),
                           np.asarray(T, dtype=np.float32))
    res = bass_utils.run_bass_kernel_spmd(nc, in_maps,
                                          core_ids=list(range(NCORES)),
                                          trace=_trace)
    out = _assemble(res.results)
    if _trace:
        return out, res
    return out


# revision 24
# speedup vs baseline: 1.0089x; 1.0089x over previous
"""Trainium2 Bass kernel for MinibatchDiscrimination.

Math (reference):
    M = (x @ T.reshape(512, 320)).reshape(1024, 64, 5)
    dist[i, j, f] = sum_k |M[i, f, k] - M[j, f, k]|
    out[i, f] = sum_j exp(-dist[i, j, f])            # (1024, 64)

Device strategy (8 cores, SPMD, rows of the output sharded):
  Every core holds MT = (T2r)^T @ x^T laid out as (320 rows = k*64+f,
  1024 cols = j) in SBUF (bf16), where its own 128 output rows are MT
  columns 0..127 (host rotates x's rows per core).  Per output row i:

    relu trick:  |d| = 2*relu(d) - d   with  d_k = MT_k[f, j] - MT_k[f, i]
    sum_k d_k   = SM[f, j] - SM[f, i]  with  SM = sum_k MT_k
    => dist = 2*( sum_k relu(d_k) - SM_j/2 ) - (-SM_i)

    - DVE:  3x tensor_scalar (subtract, max 0) over the three 128-row
      chunks of MT -> relu tiles (bf16, 4x perf mode).
    - PE:   selection-matmul k-sum of the relu tiles into one PSUM bank
      (128 = (f, j-half) partitions x 512) + one identity matmul
      streaming the static -SM/2 plane.
    - ACT:  exp(-2*PSUM + bias=-SM_i) with accum_out = j-sum -> one
      column of the (128, 128) output accumulator.

  Prologue engineering (the loop above is engine-floor bound; the
  measured waste was a ~30us prologue):
    - inputs DMA'd in 128-row contraction chunks across 3 queues so the
      first MT matmul starts ~1.5us in instead of waiting for the full
      transfer;
    - SM is produced by the same matmuls as the k4 MT chunk via an
      augmented stationary [T2sum | T2k4] (host ships T2sum = sum_k
      T2_k), instead of 6 extra selection matmuls;
    - per-i scalar tables (mts*, negsm) are built only for the 128 own
      columns.

  Host: out[i, f] = acc[f, i] + acc[f + 64, i], gather 8 cores.
"""

import numpy as np
import ml_dtypes

import concourse.bass as bass
import concourse.bacc as bacc
import concourse.mybir as mybir
import concourse.tile as tile
from concourse import bass_utils

BF16 = ml_dtypes.bfloat16

N, IN_F, OUT_F, KD = 1024, 512, 64, 5
NCORES = 8
ROWS = N // NCORES          # 128 output rows per core
R2 = 384                    # t2all cols: 256 (k0..k3) + 64 (T2sum) + 64 (k4)
FCH = IN_F // 128           # 4 contraction chunks for the MT matmul
JH = N // 2                 # 512, j-half width (PSUM bank)

_COMPILED = None            # compile once per process


def _build_program():
    nc = bacc.Bacc("TRN2", target_bir_lowering=False, debug=False,
                   num_devices=NCORES)
    dt = mybir.dt

    xt_d = nc.dram_tensor("xt", [IN_F, N], dt.bfloat16, kind="ExternalInput").ap()
    t2_d = nc.dram_tensor("t2a", [IN_F, R2], dt.bfloat16, kind="ExternalInput").ap()
    sel_d = nc.dram_tensor("sel", [128, 64], dt.bfloat16, kind="ExternalInput").ap()
    dup_d = nc.dram_tensor("dup", [64, 128], dt.bfloat16, kind="ExternalInput").ap()
    idn_d = nc.dram_tensor("idn", [128, 128], dt.bfloat16, kind="ExternalInput").ap()
    acc_d = nc.dram_tensor("acc", [128, 128], dt.float32, kind="ExternalOutput").ap()

    with tile.TileContext(nc) as tc:
        with (
            tc.tile_pool(name="persist", bufs=1) as pp,
            tc.tile_pool(name="relu", bufs=10) as rp,
            tc.tile_pool(name="psA", bufs=2, space="PSUM") as psA,
            tc.tile_pool(name="psB", bufs=6, space="PSUM") as psB,
        ):
            # ---- chunked input DMAs (3 queues) so PE starts early ----------
            xt_big = pp.tile([128, FCH * N], dt.bfloat16, tag="xtb", name="xt_big")
            t2_big = pp.tile([128, FCH * R2], dt.bfloat16, tag="t2b", name="t2_big")
            xt_sb = [xt_big[:, fc * N:(fc + 1) * N] for fc in range(FCH)]
            t2_sb = [t2_big[:, fc * R2:(fc + 1) * R2] for fc in range(FCH)]
            for fc in range(FCH):
                psl = slice(fc * 128, (fc + 1) * 128)
                q = nc.sync if fc % 2 == 0 else nc.gpsimd
                q.dma_start(xt_sb[fc], xt_d[psl, :])
                q.dma_start(t2_sb[fc], t2_d[psl, :])
            sel_sb = pp.tile([128, 64], dt.bfloat16, tag="sel", name="sel_sb")
            nc.gpsimd.dma_start(sel_sb[:], sel_d[:])
            dup_sb = pp.tile([64, 128], dt.bfloat16, tag="dup", name="dup_sb")
            nc.gpsimd.dma_start(dup_sb[:], dup_d[:])
            idn_sb = pp.tile([128, 128], dt.bfloat16, tag="idn", name="idn_sb")
            nc.gpsimd.dma_start(idn_sb[:], idn_d[:])

            # ---- MT = t2^T @ xt  (320, 1024) in 3 chunk tiles, bf16 ---------
            # chunk 0: rows 0..127 (k0,k1), chunk 1: 128..255 (k2,k3),
            # k4 packed as (f, j-half) x 512 (mtb2p).  SM rides along in the
            # k4 matmuls via the [T2sum | T2k4] stationary columns 256..384.
            mtb = [
                pp.tile([128, N], dt.bfloat16, tag="mtb0", name="mtb0"),
                pp.tile([128, N], dt.bfloat16, tag="mtb1", name="mtb1"),
            ]
            mtb2p = pp.tile([128, JH], dt.bfloat16, tag="mtb2p", name="mtb2p")
            smb = pp.tile([64, N], dt.bfloat16, tag="smb", name="smb")
            stage = pp.tile([128, JH], dt.bfloat16, tag="stage", name="stage")
            # fp32 per-i scalar tables; only the core's own 128 columns.
            mts = [
                pp.tile([128, ROWS], dt.float32, tag="mts0", name="mts0"),
                pp.tile([128, ROWS], dt.float32, tag="mts1", name="mts1"),
            ]
            mts2p = pp.tile([128, ROWS], dt.float32, tag="mts2p", name="mts2p")

            # rc-waves, fc-inner so each wave streams chunks as they land.
            for rc in (0, 1):
                rsl = slice(rc * 128, rc * 128 + 128)
                pss = [psA.tile([128, JH], dt.float32, tag="psA", name="psA")
                       for _ in range(2)]
                for fc in range(FCH):
                    for h in range(2):
                        nc.tensor.matmul(
                            pss[h][:], lhsT=t2_sb[fc][:, rsl],
                            rhs=xt_sb[fc][:, h * JH:(h + 1) * JH],
                            start=(fc == 0), stop=(fc == FCH - 1),
                        )
                for h in range(2):
                    nc.scalar.copy(mtb[rc][:, h * JH:(h + 1) * JH], pss[h][:])
                nc.vector.tensor_copy(mts[rc][:], mtb[rc][:, 0:ROWS])

            # k4 + SM waves: stationary cols 256:384 = [T2sum | T2k4]:
            # out partitions 0..63 = SM, 64..127 = M_k4.
            for h in range(2):
                jsl = slice(h * JH, (h + 1) * JH)
                ps = psA.tile([128, JH], dt.float32, tag="psA", name="psA")
                for fc in range(FCH):
                    nc.tensor.matmul(ps[:], lhsT=t2_sb[fc][:, 256:384],
                                     rhs=xt_sb[fc][:, jsl],
                                     start=(fc == 0), stop=(fc == FCH - 1))
                nc.scalar.copy(smb[:, jsl], ps[0:64, :])
                if h == 0:
                    nc.scalar.copy(stage[64:128, :], ps[64:128, :])
                else:
                    nc.scalar.copy(mtb2p[64:128, :], ps[64:128, :])
            # partition shift 64..127 -> 0..63 for the h0 k4 plane (DMA can
            # cross partitions; engines cannot).
            nc.sync.dma_start(mtb2p[0:64, :], stage[64:128, :])

            # mts2p[f + 64h, i] = mtb2p[f, i] for both h (exact upcast via
            # the 64->128 duplicator matmul; bf16 -> fp32 is exact).
            psd = psA.tile([128, JH], dt.float32, tag="psA", name="psA")
            nc.tensor.matmul(psd[:, 0:ROWS], lhsT=dup_sb[:],
                             rhs=mtb2p[0:64, 0:ROWS], start=True, stop=True)
            nc.scalar.copy(mts2p[:], psd[:, 0:ROWS])

            # ---- -SM/2 packed (f, h) + per-i bias, both smb-derived so the
            # self-term cancels exactly (bf16(-x/2) and fp32(-x) are exact).
            smp = pp.tile([128, JH], dt.bfloat16, tag="smp", name="smp")
            negsm = pp.tile([128, ROWS], dt.float32, tag="negsm", name="negsm")
            for h in range(2):
                jsl = slice(h * JH, (h + 1) * JH)
                ps = psA.tile([128, JH], dt.float32, tag="psA", name="psA")
                nc.tensor.matmul(ps[:], lhsT=dup_sb[:], rhs=smb[:, jsl],
                                 start=True, stop=True)
                nc.scalar.mul(smp[h * 64:h * 64 + 64, :],
                              ps[h * 64:h * 64 + 64, :], -0.5)
                if h == 0:
                    nc.scalar.activation(negsm[:], ps[:, 0:ROWS],
                                         mybir.ActivationFunctionType.Copy,
                                         bias=0.0, scale=-1.0)

            # ---- output accumulator + ACT scratch ---------------------------
            outacc = pp.tile([128, ROWS], dt.float32, tag="outacc", name="outacc")
            esc = psA.tile([128, JH], dt.float32, tag="psA", name="psA")



            # ---- main loop over the core's 128 output rows ------------------
            for i in range(ROWS):
                r0 = rp.tile([128, N], dt.bfloat16, tag="r0", name="r0")
                r1 = rp.tile([128, N], dt.bfloat16, tag="r1", name="r1")
                c2tt = rp.tile([128, JH], dt.bfloat16, tag="c2t", name="c2t")
                c2t = c2tt[:]
                nc.vector.tensor_scalar(
                    out=r0[:], in0=mtb[0][:], scalar1=mts[0][:, i:i + 1],
                    scalar2=0.0, op0=mybir.AluOpType.subtract,
                    op1=mybir.AluOpType.max)
                nc.vector.tensor_scalar(
                    out=r1[:], in0=mtb[1][:], scalar1=mts[1][:, i:i + 1],
                    scalar2=0.0, op0=mybir.AluOpType.subtract,
                    op1=mybir.AluOpType.max)
                nc.vector.tensor_scalar(
                    out=c2t, in0=mtb2p[:], scalar1=mts2p[:, i:i + 1],
                    scalar2=0.0, op0=mybir.AluOpType.subtract,
                    op1=mybir.AluOpType.max)

                # one PSUM bank, partitions (f, h): per-h selection-matmul
                # groups on disjoint partition ranges, then two full-height
                # identity matmuls add the packed k4 relu and the static
                # -SM/2.  The sim's flat group-check conflates the per-h
                # groups; pending-zero semantics stay per-partition exact.
                ps = psB.tile([128, JH], dt.float32, tag="psB", name="psB")
                for h in range(2):
                    jsl = slice(h * JH, (h + 1) * JH)
                    osl = ps[h * 64:h * 64 + 64, :]
                    nc.tensor.matmul(osl, lhsT=sel_sb[:], rhs=r0[:, jsl],
                                     start=True, stop=False,
                                     skip_group_check=True)
                    nc.tensor.matmul(osl, lhsT=sel_sb[:], rhs=r1[:, jsl],
                                     start=False, stop=False,
                                     skip_group_check=True)
                nc.tensor.matmul(ps[:], lhsT=idn_sb[:], rhs=c2t,
                                 start=False, stop=False,
                                 skip_group_check=True)
                nc.tensor.matmul(ps[:], lhsT=idn_sb[:], rhs=smp[:],
                                 start=False, stop=True,
                                 skip_group_check=True)

                nc.scalar.activation(
                    esc[:], ps[:], mybir.ActivationFunctionType.Exp,
                    bias=negsm[:, i:i + 1], scale=-2.0,
                    accum_out=outacc[:, i:i + 1])

            nc.sync.dma_start(acc_d[:], outacc[:])

    nc.compile()
    return nc


def _host_inputs(x, T):
    """Full-input host prep -> per-core input maps."""
    xt = np.ascontiguousarray(x.T).astype(BF16)                  # (512, 1024)
    t2r = np.ascontiguousarray(
        T.transpose(0, 2, 1).reshape(IN_F, OUT_F * KD)).astype(BF16)
    t2sum = (t2r[:, 0:64].astype(np.float32) + t2r[:, 64:128]
             + t2r[:, 128:192] + t2r[:, 192:256]
             + t2r[:, 256:320]).astype(BF16)
    t2a = np.concatenate([t2r[:, 0:256], t2sum, t2r[:, 256:320]], axis=1)

    f_idx = np.arange(64)
    sel = (np.arange(128)[:, None] % 64 == f_idx[None, :]).astype(BF16)
    dup = (np.arange(128)[None, :] % 64 == np.arange(64)[:, None]).astype(BF16)
    idn = np.eye(128, dtype=np.float32).astype(BF16)

    in_maps = []
    for c in range(NCORES):
        xt_c = np.roll(xt, -ROWS * c, axis=1)
        in_maps.append({"xt": np.ascontiguousarray(xt_c),
                        "t2a": np.ascontiguousarray(t2a),
                        "sel": sel, "dup": dup, "idn": idn})
    return in_maps


def _assemble(results):
    outs = []
    for c in range(NCORES):
        acc = results[c]["acc"]                      # (128, 128) f32
        outs.append((acc[:64, :] + acc[64:, :]).T)   # (128 rows, 64 f)
    return np.ascontiguousarray(np.concatenate(outs, axis=0), dtype=np.float32)


def _ensure_ntff_hook():
    """The agent image's antenv lacks axon_hooks; shim it so trace=True
    works (bass_utils imports antenv.axon_hooks unconditionally)."""
    import sys
    import types
    try:
        from antenv import axon_hooks  # noqa: F401
        return
    except ImportError:
        pass
    mod = types.ModuleType("antenv.axon_hooks")
    holder = [None]
    mod.set_axon_ntff_profile_hook = lambda h: holder.__setitem__(0, h)
    mod.get_axon_ntff_profile_hook = lambda: holder[0]
    import antenv
    antenv.axon_hooks = mod
    sys.modules["antenv.axon_hooks"] = mod
    try:
        from trn_agent_boot.trn_boot import _ntff_profile_via_ctypes
        h = _ntff_profile_via_ctypes("/opt/axon/libaxon_pjrt.so")
        if h is not None:
            mod.set_axon_ntff_profile_hook(h)
    except Exception:
        pass


def _get_compiled():
    global _COMPILED
    if _COMPILED is None:
        _COMPILED = _build_program()
    return _COMPILED


def kernel(x, T, _trace=False):
    if _trace:
        _ensure_ntff_hook()
    nc = _get_compiled()
    in_maps = _host_inputs(np.asarray(x, dtype=np.float32),
                           np.asarray(T, dtype=np.float32))
    res = bass_utils.run_bass_kernel_spmd(nc, in_maps,
                                          core_ids=list(range(NCORES)),
                                          trace=_trace)
    out = _assemble(res.results)
    if _trace:
        return out, res
    return out
